# revision 2
# baseline (speedup 1.0000x reference)
"""Multi-head graph attention kernel v2 for Trainium2 (8 NeuronCores).

Problem: B=8, N=1024, F_IN=F_OUT=128, H=8, D_K=16, sparse 0/1 adjacency mask.
Sharding: data-parallel over B - core b processes batch element b.

v2 vs v1 (measured v1: 133us/kernel; PE is SERIAL per matmul, no tile
concurrency, so PE cost ~ total streamed columns; ACT exp 64x[128,1024]
is ~66us busy; the rowsum DMA-roundtrip burned ~25us of DGE time):
  - NO mask-injection matmuls (was 128 MMs = 27.5us PE).  The mask is a
    0/1 bf16 tensor multiplied into P after exp (DVE 2x bf16), or folded
    into the DVE Schraudolph-exp path.
  - exp is SPLIT between ScalarE (exact exp on raw scores) and VectorE
    (Schraudolph bit-trick exp: i16 = round(S*a + b); bitcast to bf16 is
    2^(i/128 - 127) ~ exp(S/4) within ~3.5%), tunable per (g,qh,j) group.
  - rowsum normalization via stream_shuffle partition-broadcast +
    reciprocal + one multiply (no DRAM roundtrip).  vaug is padded to 32
    columns of zeros so AV matmuls write full 32-row slabs; pad rows of
    otn come out exactly 0 (0 * finite), keeping phase 3 clean.

Math (identical to reference up to fp rounding):
    Q = X@Wq.T + bq ; K = X@Wk.T + bk ; V = X@Wv.T
    S = Q_h @ K_h.T   (raw, unmasked; |S|<~12)
    P = exp(S/4) * A  (A in {0,1}; softmax shift-invariance makes the
    max-subtract unnecessary; masked_fill(-1e9)+softmax == A-weighted)
    out = (P @ V_h) / rowsum(P) @ Wo.T + (bo + Wo@bv)
"""

import sys

sys.path.insert(0, "/opt/trn_rl_repo")

import numpy as np
import ml_dtypes

BF16 = ml_dtypes.bfloat16

B, N, C, F, H, D = 8, 1024, 128, 128, 8, 16
NB = N // 128  # 8 k-blocks

LOG2E = 1.4426950408889634
SCH_C = 0.04303  # Schraudolph correction
SCH_A = 0.25 * LOG2E * 128.0
SCH_B = (127.0 - SCH_C) * 128.0

# Per (g, qh, j) group mode: "act" = ScalarE exact exp + DVE mask-mult;
# "dve" = VectorE Schraudolph (tensor_scalar->i16, bitcast, mask-mult);
# "gps" = like "act" but the mask-mult runs on GpSimd.
# 32 groups keyed (g, qh, j).
MODE = {}
for _g in range(2):
    for _qh in range(2):
        for _j in range(NB):
            MODE[(_g, _qh, _j)] = "act"
# starting split: 3 dve + 5 gps groups (tuned by measurement)
for _k in [(0, 0, 1), (1, 1, 6), (0, 1, 3)]:
    MODE[_k] = "dve"
for _k in [(1, 0, 5), (0, 0, 6), (1, 1, 2), (0, 1, 7), (1, 0, 0)]:
    MODE[_k] = "gps"

_CACHED = None


def _split_multi_waits(nc):
    """This toolchain's walrus accepts at most ONE sync wait per instruction.
    Tile emits several; split the extras onto preceding same-engine NOPs."""
    import concourse.mybir as mybir

    for f in nc.m.functions:
        for blk in f.blocks:
            new = []
            for inst in blk.instructions:
                si = inst.sync_info
                if si is not None and si.on_wait is not None and len(si.on_wait) > 1:
                    waits = list(si.on_wait)
                    for w in waits[:-1]:
                        nop = mybir.InstNoOp(
                            name=nc.get_next_instruction_name(), ins=[], outs=[])
                        nop.engine = inst.engine
                        nop.sync_info = mybir.SyncInfo(on_wait=[w], on_update=[])
                        new.append(nop)
                    inst.sync_info = mybir.SyncInfo(
                        on_wait=[waits[-1]], on_update=list(si.on_update or []))
                new.append(inst)
            del blk.instructions[:]
            for i in new:
                blk.instructions.append(i)


def _build_nc(repeat=1):
    import concourse.bass as bass
    import concourse.tile as tile
    from concourse import mybir

    f32 = mybir.dt.float32
    bf16 = mybir.dt.bfloat16
    i16 = mybir.dt.int16
    AF = mybir.ActivationFunctionType
    OP = mybir.AluOpType

    nc = bass.Bass()

    xt_d = nc.declare_dram_parameter("xt", [C, N], bf16, isOutput=False)
    at_d = nc.declare_dram_parameter("at", [N, N], bf16, isOutput=False)
    wqt_d = nc.declare_dram_parameter("wqt", [2, C, 128], bf16, isOutput=False)
    wkt_d = nc.declare_dram_parameter("wkt", [2, C, 128], bf16, isOutput=False)
    wvt_d = nc.declare_dram_parameter("wvt", [C, F], bf16, isOutput=False)
    wot_d = nc.declare_dram_parameter("wot", [2, 128, F], bf16, isOutput=False)
    bq_d = nc.declare_dram_parameter("bq2", [2, 128, 1], f32, isOutput=False)
    bk_d = nc.declare_dram_parameter("bk2", [2, 128, 1], f32, isOutput=False)
    bfin_d = nc.declare_dram_parameter("bfin", [F, 1], f32, isOutput=False)
    yt_d = nc.declare_dram_parameter("yt", [F, N], f32, isOutput=True)

    with tile.TileContext(nc) as tc:
        with tc.tile_pool(name="consts", bufs=1) as cp:
            xt_sb = cp.tile([C, N], bf16, name="xt_sb")
            nc.sync.dma_start(out=xt_sb[:], in_=xt_d[:, :])

            at_sb = []
            for j in range(NB):
                t = cp.tile([128, N], bf16, name=f"at_sb{j}")
                nc.sync.dma_start(out=t[:], in_=at_d[j * 128 : (j + 1) * 128, :])
                at_sb.append(t)

            wq_sb, wk_sb, wo_sb, bq_sb, bk_sb = [], [], [], [], []
            for g in range(2):
                w = cp.tile([C, 128], bf16, name=f"wq_sb{g}")
                nc.sync.dma_start(out=w[:], in_=wqt_d[g, :, :])
                wq_sb.append(w)
                w = cp.tile([C, 128], bf16, name=f"wk_sb{g}")
                nc.sync.dma_start(out=w[:], in_=wkt_d[g, :, :])
                wk_sb.append(w)
                w = cp.tile([128, F], bf16, name=f"wo_sb{g}")
                nc.sync.dma_start(out=w[:], in_=wot_d[g, :, :])
                wo_sb.append(w)
                b = cp.tile([128, 1], f32, name=f"bq_sb{g}")
                nc.sync.dma_start(out=b[:], in_=bq_d[g, :, :])
                bq_sb.append(b)
                b = cp.tile([128, 1], f32, name=f"bk_sb{g}")
                nc.sync.dma_start(out=b[:], in_=bk_d[g, :, :])
                bk_sb.append(b)
            wv_sb = cp.tile([C, F], bf16, name="wv_sb")
            nc.sync.dma_start(out=wv_sb[:], in_=wvt_d[:, :])
            bfin_sb = cp.tile([F, 1], f32, name="bfin_sb")
            nc.sync.dma_start(out=bfin_sb[:], in_=bfin_d[:, :])

            # V augmented per head to 32 columns: [k, j, h, 32]; col D (=16)
            # is the ones column (rowsum), cols 17..31 stay zero so the AV
            # matmuls write full 32-row slabs (keeps PSUM pad rows finite 0).
            vaug_sb = cp.tile([128, NB, H, 32], bf16, name="vaug_sb")
            nc.vector.memset(vaug_sb[:], 0.0)
            nc.vector.memset(vaug_sb[:, :, :, D : D + 1], 1.0)

            # Per-group normalized head outputs OT[hd, q] (full 128 rows
            # written every iteration; pad rows are exact 0).
            otn_sb = [cp.tile([128, N], bf16, name=f"otn_sb{g}") for g in range(2)]

            yt_sb = cp.tile([F, N], f32, name="yt_sb")

            # QT/KT per group, head a at partitions 32a..32a+15 (16..31 zero)
            qt_sb = [cp.tile([128, N], bf16, name=f"qt_sb{g}") for g in range(2)]
            kt_sb = [cp.tile([128, N], bf16, name=f"kt_sb{g}") for g in range(2)]

            def emit():
                # ---------------- Phase 1: projections ----------------
                with tc.tile_pool(name="proj_ps", bufs=2, space="PSUM") as pp:
                    for g in range(2):
                        qps = pp.tile([128, N], f32, tag="qkps")
                        nc.tensor.matmul(qps[:, 0:512], lhsT=wq_sb[g][:], rhs=xt_sb[:, 0:512])
                        nc.tensor.matmul(qps[:, 512:N], lhsT=wq_sb[g][:], rhs=xt_sb[:, 512:N])
                        nc.vector.tensor_scalar_add(qt_sb[g][:], qps[:], bq_sb[g][:])
                        kps = pp.tile([128, N], f32, tag="qkps")
                        nc.tensor.matmul(kps[:, 0:512], lhsT=wk_sb[g][:], rhs=xt_sb[:, 0:512])
                        nc.tensor.matmul(kps[:, 512:N], lhsT=wk_sb[g][:], rhs=xt_sb[:, 512:N])
                        nc.vector.tensor_scalar_add(kt_sb[g][:], kps[:], bk_sb[g][:])
                    # V projection: all 8 k-blocks into one 2-bank psum tile,
                    # one strided evacuation copy into vaug.
                    vps = pp.tile([128, NB, F], f32, tag="vps")
                    for j in range(NB):
                        nc.tensor.matmul(vps[:, j, :],
                                         lhsT=xt_sb[:, j * 128 : (j + 1) * 128],
                                         rhs=wv_sb[:])
                    nc.vector.tensor_copy(
                        out=vaug_sb[:, :, :, 0:D],
                        in_=vps[:].rearrange("p j (h d) -> p j h d", d=D))

                # ---------------- Phase 2: attention ----------------
                with (
                    tc.tile_pool(name="s_ps", bufs=3, space="PSUM") as sp,
                    tc.tile_pool(name="ov_ps", bufs=2, space="PSUM") as op_,
                    tc.tile_pool(name="ptp", bufs=6) as ptp,
                    tc.tile_pool(name="smalls", bufs=3) as smp,
                ):
                    for g in range(2):
                        for qh in range(2):
                            q0 = qh * 512
                            ov = op_.tile([128, 512], f32, tag="ov")
                            def at_rep(j, q0):
                                # at[j][:, q0:q0+512] repeated twice along the
                                # free dim (both heads of a pair share it)
                                base = at_sb[j][:, q0 : q0 + 512]
                                return bass.AP(tensor=base.tensor,
                                               offset=base.offset,
                                               ap=[[N, 128], [0, 2], [1, 512]])

                            for j in range(NB):
                                mode = MODE[(g, qh, j)]
                                spss = []
                                for pair in range(2):
                                    sps = sp.tile([128, 1024], f32, tag="s")
                                    for ai in range(2):
                                        a = pair * 2 + ai
                                        nc.tensor.matmul(
                                            sps[:, ai * 512 : (ai + 1) * 512],
                                            lhsT=kt_sb[g][32 * a : 32 * a + 32,
                                                          j * 128 : (j + 1) * 128],
                                            rhs=qt_sb[g][32 * a : 32 * a + 32,
                                                         q0 : q0 + 512],
                                            start=True, stop=True,
                                            skip_group_check=True,
                                            tile_position=(32 * a, 0),
                                        )
                                    spss.append(sps)
                                pts = []
                                for pair in range(2):
                                    # mask columns for this pair's two heads:
                                    # head a covers q-slice ai*512 within the
                                    # tile, mask cols are q0+ai*512 of at[j]
                                    if mode == "dve":
                                        # Schraudolph: i16 = rint(S*a + b)
                                        itile = ptp.tile([128, 1024], i16, tag="pt")
                                        nc.vector.tensor_scalar(
                                            out=itile[:], in0=spss[pair][:],
                                            scalar1=float(SCH_A), scalar2=float(SCH_B),
                                            op0=OP.mult, op1=OP.add)
                                        pt = ptp.tile([128, 1024], bf16, tag="pt")
                                        nc.vector.tensor_tensor(
                                            out=pt[:], in0=itile[:].bitcast(bf16),
                                            in1=at_rep(j, q0), op=OP.mult)
                                        pts.append(pt)
                                    else:
                                        pr = ptp.tile([128, 1024], bf16, tag="pt")
                                        nc.scalar.activation(out=pr[:], in_=spss[pair][:],
                                                             func=AF.Exp, scale=0.25)
                                        pt = ptp.tile([128, 1024], bf16, tag="pt")
                                        eng = nc.gpsimd if mode == "gps" else nc.vector
                                        eng.tensor_tensor(
                                            out=pt[:], in0=pr[:],
                                            in1=at_rep(j, q0), op=OP.mult)
                                        pts.append(pt)
                                for pair in range(2):
                                    for ai in range(2):
                                        a = pair * 2 + ai
                                        nc.tensor.matmul(
                                            ov[32 * a : 32 * a + 32, :],
                                            lhsT=vaug_sb[:, j, 4 * g + a, :],
                                            rhs=pts[pair][:, ai * 512 : (ai + 1) * 512],
                                            start=(j == 0),
                                            stop=(j == NB - 1),
                                            tile_position=(0, 32 * a),
                                        )
                            # normalize: rowsums live at partition 32a+16;
                            # broadcast them across each 32-partition quadrant
                            # (stream_shuffle), reciprocal, multiply.  Pad rows
                            # of ov are exact 0 -> otn pad rows exact 0.
                            rsb = smp.tile([128, 512], f32, tag="rsb")
                            nc.vector.stream_shuffle(rsb[:], ov[:], [16] * 32)
                            rsr = smp.tile([128, 512], f32, tag="rsr")
                            nc.vector.reciprocal(out=rsr[:], in_=rsb[:])
                            nc.vector.tensor_tensor(
                                out=otn_sb[g][:, q0 : q0 + 512],
                                in0=ov[:], in1=rsr[:], op=OP.mult)

                # ---------------- Phase 3: output projection ----------------
                with tc.tile_pool(name="y_ps", bufs=2, space="PSUM") as yp:
                    for qh in range(2):
                        q0 = qh * 512
                        yps = yp.tile([F, 512], f32, tag="y")
                        nc.tensor.matmul(yps[:], lhsT=wo_sb[0][:],
                                         rhs=otn_sb[0][:, q0 : q0 + 512],
                                         start=True, stop=False)
                        nc.tensor.matmul(yps[:], lhsT=wo_sb[1][:],
                                         rhs=otn_sb[1][:, q0 : q0 + 512],
                                         start=False, stop=True)
                        nc.vector.tensor_scalar_add(yt_sb[:, q0 : q0 + 512], yps[:],
                                                    bfin_sb[:])
                nc.sync.dma_start(out=yt_d[:, :], in_=yt_sb[:])

            if repeat > 1:
                with tc.For_i(0, repeat, 1):
                    emit()
            else:
                emit()

    _split_multi_waits(nc)
    return nc


def _prep_host(inputs):
    """Host-side layout prep. Returns per-core input maps."""
    X = np.asarray(inputs["X"], dtype=np.float32)
    A = np.asarray(inputs["A"], dtype=np.float32)
    Wq = np.asarray(inputs["Wq"], dtype=np.float32)
    bq = np.asarray(inputs["bq"], dtype=np.float32)
    Wk = np.asarray(inputs["Wk"], dtype=np.float32)
    bk = np.asarray(inputs["bk"], dtype=np.float32)
    Wv = np.asarray(inputs["Wv"], dtype=np.float32)
    bv = np.asarray(inputs["bv"], dtype=np.float32)
    Wo = np.asarray(inputs["Wo"], dtype=np.float32)
    bo = np.asarray(inputs["bo"], dtype=np.float32)

    # grouped/padded QK weights: wqt[g, c, 32a+d] = Wq[(4g+a)*16+d, c], d<16
    def qk_prep(W, b):
        W4 = W.reshape(2, 4, D, C)  # [g, a, d, c]
        wt = np.zeros((2, C, 4, 32), dtype=np.float32)
        wt[:, :, :, :D] = W4.transpose(0, 3, 1, 2)
        b4 = b.reshape(2, 4, D)
        bt = np.zeros((2, 4, 32), dtype=np.float32)
        bt[:, :, :D] = b4
        return (wt.reshape(2, C, 128).astype(BF16),
                bt.reshape(2, 128, 1).astype(np.float32))

    wqt, bq2 = qk_prep(Wq, bq)
    wkt, bk2 = qk_prep(Wk, bk)
    wvt = Wv.T.copy().astype(BF16)  # [c, f]
    # wot[g, 32a+d, f] = Wo[f, (4g+a)*16+d], d<16
    Wo4 = Wo.reshape(F, 2, 4, D)  # [f, g, a, d]
    wot = np.zeros((2, 4, 32, F), dtype=np.float32)
    wot[:, :, :D, :] = Wo4.transpose(1, 2, 3, 0)
    wot = wot.reshape(2, 128, F).astype(BF16)
    bfin = (bo + Wo @ bv).reshape(F, 1).astype(np.float32)

    XT = X.transpose(0, 2, 1).astype(BF16)  # [b, c, n]
    # transposed 0/1 mask [k, q]
    AT = (A.transpose(0, 2, 1) > 0).astype(BF16)

    in_maps = []
    for b in range(B):
        in_maps.append({
            "xt": np.ascontiguousarray(XT[b]),
            "at": np.ascontiguousarray(AT[b]),
            "wqt": wqt, "wkt": wkt, "wvt": wvt, "wot": wot,
            "bq2": bq2, "bk2": bk2, "bfin": bfin,
        })
    return in_maps


def run(inputs, trace=False):
    """Returns (output [B,N,F] float32, BassKernelResults)."""
    global _CACHED
    from concourse import bass_utils

    if _CACHED is None:
        _CACHED = _build_nc()
    nc = _CACHED
    in_maps = _prep_host(inputs)
    res = bass_utils.run_bass_kernel_spmd(
        nc, in_maps, core_ids=list(range(B)), trace=trace)
    out = np.stack([np.asarray(r["yt"], dtype=np.float32).T for r in res.results])
    return out, res


def kernel(**inputs):
    out, _ = run(inputs, trace=False)
    return out


def bench_loop(inputs, R=32769, reps=5):
    """Device-side For_i repeat: per-kernel time = (wall_R - wall_1)/(R-1)."""
    import time
    from concourse import bass_utils

    in_maps = _prep_host(inputs)

    def timed(nc, reps):
        ts = []
        for _ in range(reps):
            t0 = time.perf_counter()
            bass_utils.run_bass_kernel_spmd(nc, in_maps, core_ids=list(range(B)))
            ts.append(time.perf_counter() - t0)
        return ts

    nc1 = _build_nc(1)
    ncR = _build_nc(R)
    timed(nc1, 2)  # warm both compiles
    timed(ncR, 1)
    t1s, tRs = [], []
    for _ in range(reps):
        t1s.extend(timed(nc1, 1))
        tRs.extend(timed(ncR, 1))
    t1, tR = min(t1s), min(tRs)
    per = (tR - t1) / (R - 1)
    return per, {"t1s": t1s, "tRs": tRs}


# revision 3
# speedup vs baseline: 1.1394x; 1.1394x over previous
"""Multi-head graph attention kernel v2 for Trainium2 (8 NeuronCores).

Problem: B=8, N=1024, F_IN=F_OUT=128, H=8, D_K=16, sparse 0/1 adjacency mask.
Sharding: data-parallel over B - core b processes batch element b.

v2 vs v1 (measured v1: 133us/kernel; PE is SERIAL per matmul, no tile
concurrency, so PE cost ~ total streamed columns; ACT exp 64x[128,1024]
is ~66us busy; the rowsum DMA-roundtrip burned ~25us of DGE time):
  - NO mask-injection matmuls (was 128 MMs = 27.5us PE).  The mask is a
    0/1 bf16 tensor multiplied into P after exp (DVE 2x bf16), or folded
    into the DVE Schraudolph-exp path.
  - exp is SPLIT between ScalarE (exact exp on raw scores) and VectorE
    (Schraudolph bit-trick exp: i16 = round(S*a + b); bitcast to bf16 is
    2^(i/128 - 127) ~ exp(S/4) within ~3.5%), tunable per (g,qh,j) group.
  - rowsum normalization via stream_shuffle partition-broadcast +
    reciprocal + one multiply (no DRAM roundtrip).  vaug is padded to 32
    columns of zeros so AV matmuls write full 32-row slabs; pad rows of
    otn come out exactly 0 (0 * finite), keeping phase 3 clean.

Math (identical to reference up to fp rounding):
    Q = X@Wq.T + bq ; K = X@Wk.T + bk ; V = X@Wv.T
    S = Q_h @ K_h.T   (raw, unmasked; |S|<~12)
    P = exp(S/4) * A  (A in {0,1}; softmax shift-invariance makes the
    max-subtract unnecessary; masked_fill(-1e9)+softmax == A-weighted)
    out = (P @ V_h) / rowsum(P) @ Wo.T + (bo + Wo@bv)
"""

import sys

sys.path.insert(0, "/opt/trn_rl_repo")

import numpy as np
import ml_dtypes

BF16 = ml_dtypes.bfloat16

B, N, C, F, H, D = 8, 1024, 128, 128, 8, 16
NB = N // 128  # 8 k-blocks

LOG2E = 1.4426950408889634
SCH_C = 0.04303  # Schraudolph correction
SCH_A = 0.25 * LOG2E * 128.0
SCH_B = (127.0 - SCH_C) * 128.0

# Per (g, qh, j) group mode: "act" = ScalarE exact exp + DVE mask-mult;
# "dve" = VectorE Schraudolph (tensor_scalar->i16, bitcast, mask-mult);
# "gps" = like "act" but the mask-mult runs on GpSimd.
# 32 groups keyed (g, qh, j).
MODE = {}
for _g in range(2):
    for _qh in range(2):
        for _j in range(NB):
            MODE[(_g, _qh, _j)] = "act"
# starting split: 3 dve + 5 gps groups (tuned by measurement)
import os
_cfg = os.environ.get("V3_MODES", "")
if _cfg:
    # format: "dve:g,qh,j;g,qh,j|gps:g,qh,j;..."
    for part in _cfg.split("|"):
        if not part: continue
        m, keys = part.split(":")
        for kk in keys.split(";"):
            if not kk: continue
            t = tuple(int(x) for x in kk.split(","))
            MODE[t] = m
# default: all-act (uniform ACT pacing measured fastest; dve/gps modes
# create local pipeline bubbles)
PIPE_DEPTH = int(os.environ.get("V3_DEPTH", "2"))
PTP_BUFS = int(os.environ.get("V3_PTP", "12"))
SMP_BUFS = int(os.environ.get("V3_SMP", "3"))

_CACHED = None


def _split_multi_waits(nc):
    """This toolchain's walrus accepts at most ONE sync wait per instruction.
    Tile emits several; split the extras onto preceding same-engine NOPs."""
    import concourse.mybir as mybir

    for f in nc.m.functions:
        for blk in f.blocks:
            new = []
            for inst in blk.instructions:
                si = inst.sync_info
                if si is not None and si.on_wait is not None and len(si.on_wait) > 1:
                    waits = list(si.on_wait)
                    for w in waits[:-1]:
                        nop = mybir.InstNoOp(
                            name=nc.get_next_instruction_name(), ins=[], outs=[])
                        nop.engine = inst.engine
                        nop.sync_info = mybir.SyncInfo(on_wait=[w], on_update=[])
                        new.append(nop)
                    inst.sync_info = mybir.SyncInfo(
                        on_wait=[waits[-1]], on_update=list(si.on_update or []))
                new.append(inst)
            del blk.instructions[:]
            for i in new:
                blk.instructions.append(i)


def _build_nc(repeat=1):
    import concourse.bass as bass
    import concourse.tile as tile
    from concourse import mybir

    f32 = mybir.dt.float32
    bf16 = mybir.dt.bfloat16
    i16 = mybir.dt.int16
    AF = mybir.ActivationFunctionType
    OP = mybir.AluOpType

    nc = bass.Bass()

    xt_d = nc.declare_dram_parameter("xt", [C, N], bf16, isOutput=False)
    at_d = nc.declare_dram_parameter("at", [N, N], bf16, isOutput=False)
    wqt_d = nc.declare_dram_parameter("wqt", [2, C, 128], bf16, isOutput=False)
    wkt_d = nc.declare_dram_parameter("wkt", [2, C, 128], bf16, isOutput=False)
    wvt_d = nc.declare_dram_parameter("wvt", [C, F], bf16, isOutput=False)
    wot_d = nc.declare_dram_parameter("wot", [2, 128, F], bf16, isOutput=False)
    bq_d = nc.declare_dram_parameter("bq2", [2, 128, 1], f32, isOutput=False)
    bk_d = nc.declare_dram_parameter("bk2", [2, 128, 1], f32, isOutput=False)
    bfin_d = nc.declare_dram_parameter("bfin", [F, 1], f32, isOutput=False)
    yt_d = nc.declare_dram_parameter("yt", [F, N], f32, isOutput=True)

    with tile.TileContext(nc) as tc:
        with tc.tile_pool(name="consts", bufs=1) as cp:
            xt_sb = cp.tile([C, N], bf16, name="xt_sb")
            nc.sync.dma_start(out=xt_sb[:], in_=xt_d[:, :])

            at_sb = []
            for j in range(NB):
                t = cp.tile([128, N], bf16, name=f"at_sb{j}")
                nc.sync.dma_start(out=t[:], in_=at_d[j * 128 : (j + 1) * 128, :])
                at_sb.append(t)

            wq_sb, wk_sb, wo_sb, bq_sb, bk_sb = [], [], [], [], []
            for g in range(2):
                w = cp.tile([C, 128], bf16, name=f"wq_sb{g}")
                nc.sync.dma_start(out=w[:], in_=wqt_d[g, :, :])
                wq_sb.append(w)
                w = cp.tile([C, 128], bf16, name=f"wk_sb{g}")
                nc.sync.dma_start(out=w[:], in_=wkt_d[g, :, :])
                wk_sb.append(w)
                w = cp.tile([128, F], bf16, name=f"wo_sb{g}")
                nc.sync.dma_start(out=w[:], in_=wot_d[g, :, :])
                wo_sb.append(w)
                b = cp.tile([128, 1], f32, name=f"bq_sb{g}")
                nc.sync.dma_start(out=b[:], in_=bq_d[g, :, :])
                bq_sb.append(b)
                b = cp.tile([128, 1], f32, name=f"bk_sb{g}")
                nc.sync.dma_start(out=b[:], in_=bk_d[g, :, :])
                bk_sb.append(b)
            wv_sb = cp.tile([C, F], bf16, name="wv_sb")
            nc.sync.dma_start(out=wv_sb[:], in_=wvt_d[:, :])
            bfin_sb = cp.tile([F, 1], f32, name="bfin_sb")
            nc.sync.dma_start(out=bfin_sb[:], in_=bfin_d[:, :])

            # V augmented per head to 32 columns: [k, j, h, 32]; col D (=16)
            # is the ones column (rowsum), cols 17..31 stay zero so the AV
            # matmuls write full 32-row slabs (keeps PSUM pad rows finite 0).
            vaug_sb = cp.tile([128, NB, H, 32], bf16, name="vaug_sb")
            nc.vector.memset(vaug_sb[:], 0.0)
            nc.vector.memset(vaug_sb[:, :, :, D : D + 1], 1.0)

            # Per-group normalized head outputs OT[hd, q] (full 128 rows
            # written every iteration; pad rows are exact 0).
            otn_sb = [cp.tile([128, N], bf16, name=f"otn_sb{g}") for g in range(2)]

            yt_sb = cp.tile([F, N], f32, name="yt_sb")

            # QT/KT per group, head a at partitions 32a..32a+15 (16..31 zero)
            qt_sb = [cp.tile([128, N], bf16, name=f"qt_sb{g}") for g in range(2)]
            kt_sb = [cp.tile([128, N], bf16, name=f"kt_sb{g}") for g in range(2)]

            # Prologue: projections for the first loop iteration (the loop
            # body computes NEXT-iteration projections at its tail, so the
            # steady state fully overlaps phase 1 with attention).
            with tc.tile_pool(name="prolog_ps", bufs=2, space="PSUM") as pp0:
                for g in range(2):
                    qps0 = pp0.tile([128, N], f32, tag="qkps")
                    nc.tensor.matmul(qps0[:, 0:512], lhsT=wq_sb[g][:], rhs=xt_sb[:, 0:512])
                    nc.tensor.matmul(qps0[:, 512:N], lhsT=wq_sb[g][:], rhs=xt_sb[:, 512:N])
                    nc.vector.tensor_scalar_add(qt_sb[g][:], qps0[:], bq_sb[g][:])
                    kps0 = pp0.tile([128, N], f32, tag="qkps")
                    nc.tensor.matmul(kps0[:, 0:512], lhsT=wk_sb[g][:], rhs=xt_sb[:, 0:512])
                    nc.tensor.matmul(kps0[:, 512:N], lhsT=wk_sb[g][:], rhs=xt_sb[:, 512:N])
                    nc.vector.tensor_scalar_add(kt_sb[g][:], kps0[:], bk_sb[g][:])
                vps0 = pp0.tile([128, NB, F], f32, tag="vps")
                for j in range(NB):
                    nc.tensor.matmul(vps0[:, j, :],
                                     lhsT=xt_sb[:, j * 128 : (j + 1) * 128],
                                     rhs=wv_sb[:])
                nc.vector.tensor_copy(
                    out=vaug_sb[:, :, :, 0:D],
                    in_=vps0[:].rearrange("p j (h d) -> p j h d", d=D))

            def emit():
                # ------- One unified scope: attention + out-proj + next-iter
                # projections, software-pipelined; zero pool transitions. -------
                # Tile keeps per-engine PROGRAM ORDER, so the in-order PE
                # would stall at AV(j) waiting on exp/mult(j) if emitted
                # naively.  Emit scores DEPTH groups ahead so the PE always
                # has queued work while ACT/DVE process earlier tiles.
                with (
                    tc.tile_pool(name="s_ps", bufs=3, space="PSUM") as sp,
                    tc.tile_pool(name="ov_ps", bufs=2, space="PSUM") as op_,
                    tc.tile_pool(name="ptp", bufs=PTP_BUFS) as ptp,
                    tc.tile_pool(name="smalls", bufs=SMP_BUFS) as smp,
                ):
                    def at_rep(j, q0):
                        # at[j][:, q0:q0+512] repeated twice along the free
                        # dim (both heads of a pair share it)
                        base = at_sb[j][:, q0 : q0 + 512]
                        return bass.AP(tensor=base.tensor,
                                       offset=base.offset,
                                       ap=[[N, 128], [0, 2], [1, 512]])

                    order = [(g, qh, j)
                             for g in range(2) for qh in range(2)
                             for j in range(NB)]
                    DEPTH = PIPE_DEPTH
                    score_tiles = {}
                    ov_tiles = {}

                    def emit_scores(g, qh, j):
                        q0 = qh * 512
                        spss = []
                        for pair in range(2):
                            sps = sp.tile([128, 1024], f32, tag="s")
                            for ai in range(2):
                                a = pair * 2 + ai
                                nc.tensor.matmul(
                                    sps[:, ai * 512 : (ai + 1) * 512],
                                    lhsT=kt_sb[g][32 * a : 32 * a + 32,
                                                  j * 128 : (j + 1) * 128],
                                    rhs=qt_sb[g][32 * a : 32 * a + 32,
                                                 q0 : q0 + 512],
                                    start=True, stop=True,
                                    skip_group_check=True,
                                    tile_position=(32 * a, 0),
                                )
                            spss.append(sps)
                        score_tiles[(g, qh, j)] = spss

                    for i in range(DEPTH):
                        emit_scores(*order[i])

                    for i, (g, qh, j) in enumerate(order):
                        q0 = qh * 512
                        if j == 0:
                            ov_t = op_.tile([128, 512], f32, tag="ov", name=f"ov_{g}_{qh}")
                            ov_tiles[(g, qh)] = ov_t
                        ov = ov_tiles[(g, qh)]
                        if i + DEPTH < len(order):
                            emit_scores(*order[i + DEPTH])
                        mode = MODE[(g, qh, j)]
                        spss = score_tiles.pop((g, qh, j))
                        pts = []
                        for pair in range(2):
                            if mode == "dve":
                                # Schraudolph: i16 = rint(S*a + b); bitcast
                                itile = ptp.tile([128, 1024], i16, tag="pt")
                                nc.vector.tensor_scalar(
                                    out=itile[:], in0=spss[pair][:],
                                    scalar1=float(SCH_A), scalar2=float(SCH_B),
                                    op0=OP.mult, op1=OP.add)
                                pt = ptp.tile([128, 1024], bf16, tag="pt")
                                nc.vector.tensor_tensor(
                                    out=pt[:], in0=itile[:].bitcast(bf16),
                                    in1=at_rep(j, q0), op=OP.mult)
                                pts.append(pt)
                            else:
                                pr = ptp.tile([128, 1024], bf16, tag="pt")
                                nc.scalar.activation(out=pr[:], in_=spss[pair][:],
                                                     func=AF.Exp, scale=0.25)
                                pt = ptp.tile([128, 1024], bf16, tag="pt")
                                eng = nc.gpsimd if mode == "gps" else nc.vector
                                eng.tensor_tensor(
                                    out=pt[:], in0=pr[:],
                                    in1=at_rep(j, q0), op=OP.mult)
                                pts.append(pt)
                        for pair in range(2):
                            for ai in range(2):
                                a = pair * 2 + ai
                                nc.tensor.matmul(
                                    ov[32 * a : 32 * a + 32, :],
                                    lhsT=vaug_sb[:, j, 4 * g + a, :],
                                    rhs=pts[pair][:, ai * 512 : (ai + 1) * 512],
                                    start=(j == 0),
                                    stop=(j == NB - 1),
                                    tile_position=(0, 32 * a),
                                )
                        if j == NB - 1:
                            # normalize: rowsums live at partition 32a+16;
                            # broadcast across each 32-partition quadrant
                            # (stream_shuffle), reciprocal, multiply.  Pad
                            # rows of ov are exact 0 -> otn pad rows exact 0.
                            rsb = smp.tile([128, 512], f32, tag="rsb")
                            nc.vector.stream_shuffle(rsb[:], ov[:], [16] * 32)
                            rsr = smp.tile([128, 512], f32, tag="rsr")
                            nc.vector.reciprocal(out=rsr[:], in_=rsb[:])
                            nc.vector.tensor_tensor(
                                out=otn_sb[g][:, q0 : q0 + 512],
                                in0=ov[:], in1=rsr[:], op=OP.mult)
                            if g == 1:
                                # out-projection for this q-half as soon as
                                # both groups' otn slices are normalized; the
                                # yps tile reuses the ov ring (1 bank).
                                yps = op_.tile([F, 512], f32, tag="ov",
                                               name=f"yps_{qh}")
                                nc.tensor.matmul(yps[:],
                                                 lhsT=wo_sb[0][:],
                                                 rhs=otn_sb[0][:, q0 : q0 + 512],
                                                 start=True, stop=False)
                                nc.tensor.matmul(yps[:],
                                                 lhsT=wo_sb[1][:],
                                                 rhs=otn_sb[1][:, q0 : q0 + 512],
                                                 start=False, stop=True)
                                nc.vector.tensor_scalar_add(
                                    yt_sb[:, q0 : q0 + 512], yps[:], bfin_sb[:])
                                nc.sync.dma_start(out=yt_d[:, q0 : q0 + 512],
                                                  in_=yt_sb[:, q0 : q0 + 512])

                    # Tail: projections for the NEXT iteration, absorbed into
                    # the pipeline (PE runs them under the last groups' exp/
                    # mult/norm work).  qps/kps ride the s ring; the two vps
                    # halves ride the ov ring.  The ring WAR semaphores plus
                    # per-engine program order transitively enforce the
                    # loop-carried RAW (tail writes -> next head reads).
                    for g in range(2):
                        qps = sp.tile([128, N], f32, tag="s", name=f"qps_{g}")
                        nc.tensor.matmul(qps[:, 0:512], lhsT=wq_sb[g][:],
                                         rhs=xt_sb[:, 0:512])
                        nc.tensor.matmul(qps[:, 512:N], lhsT=wq_sb[g][:],
                                         rhs=xt_sb[:, 512:N])
                        nc.vector.tensor_scalar_add(qt_sb[g][:], qps[:], bq_sb[g][:])
                        kps = sp.tile([128, N], f32, tag="s", name=f"kps_{g}")
                        nc.tensor.matmul(kps[:, 0:512], lhsT=wk_sb[g][:],
                                         rhs=xt_sb[:, 0:512])
                        nc.tensor.matmul(kps[:, 512:N], lhsT=wk_sb[g][:],
                                         rhs=xt_sb[:, 512:N])
                        nc.vector.tensor_scalar_add(kt_sb[g][:], kps[:], bk_sb[g][:])
                    for half in range(2):
                        vps = op_.tile([128, 4 * F], f32, tag="ov",
                                       name=f"vps_{half}")
                        for jj in range(4):
                            j = half * 4 + jj
                            nc.tensor.matmul(vps[:, jj * F : (jj + 1) * F],
                                             lhsT=xt_sb[:, j * 128 : (j + 1) * 128],
                                             rhs=wv_sb[:])
                        nc.vector.tensor_copy(
                            out=vaug_sb[:, half * 4 : half * 4 + 4, :, 0:D],
                            in_=vps[:].rearrange("p (j h d) -> p j h d",
                                                 h=H, d=D))

            if repeat > 1:
                with tc.For_i(0, repeat, 1):
                    emit()
            else:
                emit()

    _split_multi_waits(nc)
    return nc


def _prep_host(inputs):
    """Host-side layout prep. Returns per-core input maps."""
    X = np.asarray(inputs["X"], dtype=np.float32)
    A = np.asarray(inputs["A"], dtype=np.float32)
    Wq = np.asarray(inputs["Wq"], dtype=np.float32)
    bq = np.asarray(inputs["bq"], dtype=np.float32)
    Wk = np.asarray(inputs["Wk"], dtype=np.float32)
    bk = np.asarray(inputs["bk"], dtype=np.float32)
    Wv = np.asarray(inputs["Wv"], dtype=np.float32)
    bv = np.asarray(inputs["bv"], dtype=np.float32)
    Wo = np.asarray(inputs["Wo"], dtype=np.float32)
    bo = np.asarray(inputs["bo"], dtype=np.float32)

    # grouped/padded QK weights: wqt[g, c, 32a+d] = Wq[(4g+a)*16+d, c], d<16
    def qk_prep(W, b):
        W4 = W.reshape(2, 4, D, C)  # [g, a, d, c]
        wt = np.zeros((2, C, 4, 32), dtype=np.float32)
        wt[:, :, :, :D] = W4.transpose(0, 3, 1, 2)
        b4 = b.reshape(2, 4, D)
        bt = np.zeros((2, 4, 32), dtype=np.float32)
        bt[:, :, :D] = b4
        return (wt.reshape(2, C, 128).astype(BF16),
                bt.reshape(2, 128, 1).astype(np.float32))

    wqt, bq2 = qk_prep(Wq, bq)
    wkt, bk2 = qk_prep(Wk, bk)
    wvt = Wv.T.copy().astype(BF16)  # [c, f]
    # wot[g, 32a+d, f] = Wo[f, (4g+a)*16+d], d<16
    Wo4 = Wo.reshape(F, 2, 4, D)  # [f, g, a, d]
    wot = np.zeros((2, 4, 32, F), dtype=np.float32)
    wot[:, :, :D, :] = Wo4.transpose(1, 2, 3, 0)
    wot = wot.reshape(2, 128, F).astype(BF16)
    bfin = (bo + Wo @ bv).reshape(F, 1).astype(np.float32)

    XT = X.transpose(0, 2, 1).astype(BF16)  # [b, c, n]
    # transposed 0/1 mask [k, q]
    AT = (A.transpose(0, 2, 1) > 0).astype(BF16)

    in_maps = []
    for b in range(B):
        in_maps.append({
            "xt": np.ascontiguousarray(XT[b]),
            "at": np.ascontiguousarray(AT[b]),
            "wqt": wqt, "wkt": wkt, "wvt": wvt, "wot": wot,
            "bq2": bq2, "bk2": bk2, "bfin": bfin,
        })
    return in_maps


def run(inputs, trace=False):
    """Returns (output [B,N,F] float32, BassKernelResults)."""
    global _CACHED
    from concourse import bass_utils

    if _CACHED is None:
        _CACHED = _build_nc()
    nc = _CACHED
    in_maps = _prep_host(inputs)
    res = bass_utils.run_bass_kernel_spmd(
        nc, in_maps, core_ids=list(range(B)), trace=trace)
    out = np.stack([np.asarray(r["yt"], dtype=np.float32).T for r in res.results])
    return out, res


def kernel(**inputs):
    out, _ = run(inputs, trace=False)
    return out


def bench_loop(inputs, R=32769, reps=5):
    """Device-side For_i repeat: per-kernel time = (wall_R - wall_1)/(R-1)."""
    import time
    from concourse import bass_utils

    in_maps = _prep_host(inputs)

    def timed(nc, reps):
        ts = []
        for _ in range(reps):
            t0 = time.perf_counter()
            bass_utils.run_bass_kernel_spmd(nc, in_maps, core_ids=list(range(B)))
            ts.append(time.perf_counter() - t0)
        return ts

    nc1 = _build_nc(1)
    ncR = _build_nc(R)
    timed(nc1, 2)  # warm both compiles
    timed(ncR, 1)
    t1s, tRs = [], []
    for _ in range(reps):
        t1s.extend(timed(nc1, 1))
        tRs.extend(timed(ncR, 1))
    t1, tR = min(t1s), min(tRs)
    per = (tR - t1) / (R - 1)
    return per, {"t1s": t1s, "tRs": tRs}
